# revision 17
# baseline (speedup 1.0000x reference)
"""AttentionSubsample Trainium2 kernel: 8-core data-parallel over batch.

Layout strategy (per core, 4 batch elements), v3:
  - k/q/scores run in bf16 (fp8 there amplifies through the exp); v and
    the projection run as fp8e4 DoubleRow matmuls (2 K-tiles per
    instruction at 0.5 cycles/row), with x4/x16 pre-scaling so the small
    BN-folded weights stay in fp8's normal range.
  - the relative-position bias is PRELOADED into the score PSUM banks
    with fp8 DoubleRow identity matmuls (I,0)/(0,I), so exp reads (s+b)
    directly and no elementwise bias multiply exists at all.  The whole
    bias table stays resident in SBUF (one DMA), so batch output work
    spreads evenly instead of serializing at the end.
  - attention output per head pair lives in ONE psum bank (even head
    rows 0-63, odd head rows 64-127, each zeroing its rows via its
    chunk-0 start=True); softmax denominators accumulate in columns
    196..199 of the same bank via 1-row ones-matmuls (start=False: the
    attn starts already zeroed the full 2KB bank rows), so the
    denominator needs no 1-partition copy: one [98,4] reciprocal per
    pair reads it q-partitioned.
  - denominator reciprocals are transposed back to head-major [16,196]
    with a PE transpose and broadcast over partitions with a K=16 sel
    matmul; the hardswish chain is split DVE (bias add, clamp) / Pool
    (+3, multiply).
"""

import sys

sys.path.insert(0, "/opt/trn_rl_repo")

from contextlib import ExitStack

import numpy as np
import ml_dtypes

import concourse.bass as bass
import concourse.tile as tile
from concourse import bacc
from concourse import mybir
from concourse.bass_utils import run_bass_kernel_spmd

F32 = mybir.dt.float32
F32R = mybir.dt.float32r
BF16 = mybir.dt.bfloat16
FP8 = mybir.dt.float8e4
ALU = mybir.AluOpType
AF = mybir.ActivationFunctionType
DR = mybir.MatmulPerfMode.DoubleRow

FP8NP = ml_dtypes.float8_e4m3
BF16NP = ml_dtypes.bfloat16

B, N, NQ, IN, H, KD, D, OUT = 32, 784, 196, 384, 16, 32, 64, 512
HID, DH = 1536, 1024
RES, RES_, STRIDE = 28, 14, 2
SCALE = KD ** -0.5
EPS = 1e-5
NCORES = 8
BC = B // NCORES          # 4 batch elems per core
C, MC = 7, 112            # key-token chunks: 7 x 112 = 784
G, HG = 2, 8              # 2 head-groups of 8 heads

VSCALE = 4.0              # fp8 headroom scale on the v weights
PSCALE = 16.0             # fp8 headroom scale on the projection path

TRACE = False
LAST_RESULTS = None

_NC_CACHE = None


def _build_nc():
    nc = bacc.Bacc("TRN2", target_bir_lowering=False, debug=False,
                   num_devices=NCORES)

    xT = nc.dram_tensor("xT", [BC, IN, N], BF16, kind="ExternalInput").ap()
    x8T = nc.dram_tensor("x8T", [BC, 512, N], FP8, kind="ExternalInput").ap()
    xsT = nc.dram_tensor("xsT", [BC, IN, NQ], BF16, kind="ExternalInput").ap()
    wk = nc.dram_tensor("wk", [IN, 512], BF16, kind="ExternalInput").ap()
    wv = nc.dram_tensor("wv", [128, 4, DH], FP8, kind="ExternalInput").ap()
    wq = nc.dram_tensor("wq", [IN, 512], BF16, kind="ExternalInput").ap()
    wp = nc.dram_tensor("wp", [DH, OUT], BF16, kind="ExternalInput").ap()
    shk = nc.dram_tensor("shk", [128, 4], F32, kind="ExternalInput").ap()
    shq = nc.dram_tensor("shq", [128, 4], F32, kind="ExternalInput").ap()
    shv = nc.dram_tensor("shv", [128, 8], F32, kind="ExternalInput").ap()
    shp = nc.dram_tensor("shp", [1, OUT], F32R, kind="ExternalInput").ap()
    ebias = nc.dram_tensor("ebias", [MC, H, 8, NQ], FP8,
                           kind="ExternalInput").ap()
    identd = nc.dram_tensor("identd", [MC, 2, 2, MC], FP8,
                            kind="ExternalInput").ap()
    sel2d = nc.dram_tensor("sel2d", [16, 8, 128], BF16,
                           kind="ExternalInput").ap()
    idqd = nc.dram_tensor("idqd", [98, 98], F32, kind="ExternalInput").ap()
    out = nc.dram_tensor("out", [BC, NQ, OUT], F32, kind="ExternalOutput").ap()

    with tile.TileContext(nc) as tc, ExitStack() as ctx:
        ctx.enter_context(nc.allow_low_precision(
            reason="fp8/bf16 attention path validated against fp32 reference"))
        singles = ctx.enter_context(tc.tile_pool(name="singles", bufs=1))
        texpp = ctx.enter_context(tc.tile_pool(name="texpp", bufs=6))
        tmpp = ctx.enter_context(tc.tile_pool(name="tmpp", bufs=2))
        hswp = ctx.enter_context(tc.tile_pool(name="hswp", bufs=2))
        finp = ctx.enter_context(tc.tile_pool(name="finp", bufs=2))
        mmp = ctx.enter_context(tc.tile_pool(name="mmp", bufs=2, space="PSUM"))
        scp = ctx.enter_context(tc.tile_pool(name="scp", bufs=2, space="PSUM"))
        opp = ctx.enter_context(tc.tile_pool(name="opp", bufs=2, space="PSUM"))

        # --- persistent SBUF ---
        wk_sb = singles.tile([128, 3, 512], BF16)
        nc.sync.dma_start(wk_sb, wk.rearrange("(c p) n -> p c n", p=128))
        wq_sb = singles.tile([128, 3, 512], BF16)
        wv_sb = singles.tile([128, 4, DH], FP8)
        wp_sb = singles.tile([128, 8, OUT], BF16)
        shk_sb = singles.tile([128, 4], F32)
        shq_sb = singles.tile([128, 4], F32)
        shv_sb = singles.tile([128, 8], F32)
        shp_sb = singles.tile([1, OUT], F32R)
        ident_sb = singles.tile([MC, 2, 2, MC], FP8)
        sel2 = singles.tile([16, 8, 128], BF16)
        idq = singles.tile([98, 98], F32)
        ones1 = singles.tile([1, 128], F32)
        nc.gpsimd.memset(ones1, 1.0)
        ones1b = singles.tile([MC, 1], BF16)
        nc.gpsimd.memset(ones1b, 1.0)
        p3t = singles.tile([128, NQ], BF16)
        nc.gpsimd.memset(p3t, 3.0)

        # full bias table resident, loaded in per-head slices so the x
        # loads are not stuck behind one 3.5MB transfer on the DMA queue
        ebias_sb = singles.tile([MC, H, 8, NQ], FP8)

        # per-b persistent tensors
        xtb = [singles.tile([128, 3, N], BF16, name=f"xtb{b}")
               for b in range(BC)]
        x8tb = [singles.tile([128, 4, N], FP8, name=f"x8tb{b}")
                for b in range(BC)]
        xstb = [singles.tile([128, 3, NQ], BF16, name=f"xstb{b}")
                for b in range(BC)]
        kg = [singles.tile([128, 2, N], BF16, name=f"kg{b}")
              for b in range(BC)]
        qg = [singles.tile([128, 2, NQ], BF16, name=f"qg{b}")
              for b in range(BC)]
        vt = [singles.tile([MC, C, 512], BF16, name=f"vt{b}")
              for b in range(BC)]
        acc = [singles.tile([128, 8, NQ], BF16, name=f"acc{b}")
               for b in range(BC)]
        den_q = [singles.tile([98, 2, 16], F32, name=f"denq{b}")
                 for b in range(BC)]
        recsT = [singles.tile([16, NQ], BF16, name=f"recsT{b}")
                 for b in range(BC)]

        pending = None

        def emit_attn(texp2, hhs, b, g):
            t = 4 * g + hhs[0] // 2
            op = opp.tile([128, 512], F32, tag="op", name="op")
            # chunk-0 matmuls with start=True zero each head's full 2KB
            # bank rows (incl. den cols); den matmuls accumulate on that.
            for j in (0, 1):
                hh = hhs[j]
                nc.tensor.matmul(op[64 * j:64 * j + 64, 0:196],
                                 lhsT=vt[b][:, 0, 64 * hh:64 * hh + 64],
                                 rhs=texp2[j][:, 0, :],
                                 start=True, stop=False,
                                 skip_group_check=True)
            for j in (0, 1):
                for qh in (0, 1):
                    cl = 196 + 2 * j + qh
                    for c in range(C):
                        nc.tensor.matmul(
                            op[0:98, cl:cl + 1],
                            lhsT=texp2[j][:, c, 98 * qh:98 * qh + 98],
                            rhs=ones1b,
                            start=False, stop=False, skip_group_check=True)
            for j in (0, 1):
                hh = hhs[j]
                for c in range(1, C):
                    nc.tensor.matmul(op[64 * j:64 * j + 64, 0:196],
                                     lhsT=vt[b][:, c, 64 * hh:64 * hh + 64],
                                     rhs=texp2[j][:, c, :],
                                     start=False,
                                     stop=(j == 1 and c == C - 1),
                                     skip_group_check=True)
            nc.vector.tensor_copy(acc[b][:, t, :], op[:, 0:196])
            nc.vector.reciprocal(
                den_q[b][:, :, 2 * t:2 * t + 2],
                op[0:98, 196:200].rearrange("p (j q) -> p q j", j=2))

        def emit_output(b):
            # transpose the q-partitioned reciprocals to head-major [16,196]
            for qh in (0, 1):
                tr = mmp.tile([128, 512], F32, tag="mm", name="tr")
                nc.tensor.matmul(tr[0:16, 0:98],
                                 lhsT=den_q[b][0:98, qh, :], rhs=idq,
                                 is_transpose=True, start=True, stop=True)
                nc.vector.tensor_copy(recsT[b][:, 98 * qh:98 * qh + 98],
                                      tr[0:16, 0:98])
            hsw = hswp.tile([128, 8, NQ], BF16, tag="hsw", name="hsw")
            # both po banks held up front; each feature tile's projection
            # matmuls stream in as soon as its hardswish completes
            po = []
            for mt, msz in ((0, 128), (1, 68)):
                p_ = mmp.tile([128, 512], F32, tag="mm", name="po")
                nc.tensor.matmul(p_[:msz, :],
                                 lhsT=ones1.bitcast(F32R)[0:1, 0:msz],
                                 rhs=shp_sb, start=True, stop=False,
                                 skip_group_check=True)
                po.append(p_)
            for t in range(8):
                rep = opp.tile([128, 512], F32, tag="op", name="rep")
                nc.tensor.matmul(rep[:, :NQ], lhsT=sel2[:, t, :],
                                 rhs=recsT[b],
                                 start=True, stop=True,
                                 skip_group_check=True)
                t1 = tmpp.tile([128, NQ], BF16, tag="t1", name="t1")
                nc.vector.tensor_tensor(t1, acc[b][:, t, :], rep[:, :NQ],
                                        ALU.mult)
                # vv2 = max(t1 + shv, -3); t3p = min(vv2, 3) + 3
                # hsw = t3p * vv2  (exact: t3p = 0 wherever vv2 != vv)
                vv = tmpp.tile([128, NQ], BF16, tag="vv", name="vv")
                nc.vector.tensor_scalar(vv, t1, shv_sb[:, t:t + 1], -3.0,
                                        ALU.add, ALU.max)
                t3 = tmpp.tile([128, NQ], BF16, tag="t3", name="t3")
                nc.vector.tensor_scalar(t3, vv, 3.0, 3.0, ALU.min, ALU.add)
                nc.gpsimd.tensor_mul(hsw[:, t, :], t3, vv)
                for mt, msz in ((0, 128), (1, 68)):
                    nc.tensor.matmul(
                        po[mt][:msz, :],
                        lhsT=hsw[:, t, 128 * mt:128 * mt + msz],
                        rhs=wp_sb[:, t, :], start=False,
                        stop=(t == 7), skip_group_check=True)
            for mt, msz in ((0, 128), (1, 68)):
                fin = finp.tile([128, OUT], F32, tag="fin", name="fin")
                nc.vector.tensor_copy(fin[:msz, :], po[mt][:msz, :])
                nc.sync.dma_start(out[b, 128 * mt:128 * mt + msz, :],
                                  fin[:msz, :])

        def emit_phaseA(g, b):
            # ---- phase A: k, q, v for one (group, batch elem) ----
            if g == 0:
                nc.sync.dma_start(xtb[b],
                                  xT[b].rearrange("(c p) n -> p c n", p=128))
                nc.sync.dma_start(x8tb[b],
                                  x8T[b].rearrange("(c p) n -> p c n", p=128))
                nc.sync.dma_start(xstb[b],
                                  xsT[b].rearrange("(c p) n -> p c n", p=128))
            if g == 0 and b == 0:
                nc.sync.dma_start(shk_sb, shk)
                nc.sync.dma_start(ident_sb, identd)
                nc.sync.dma_start(shq_sb, shq)

            # k for this head group: features [256g, 256g+256), feat-major
            for m2 in range(2):
                for n2 in range(2):
                    pk = mmp.tile([128, 512], F32, tag="mm", name="pk")
                    for kk in range(3):
                        nc.tensor.matmul(
                            pk[:, :392],
                            lhsT=wk_sb[:, kk, 256 * g + 128 * m2:
                                       256 * g + 128 * m2 + 128],
                            rhs=xtb[b][:, kk, 392 * n2:392 * n2 + 392],
                            start=(kk == 0), stop=(kk == 2))
                    nc.vector.tensor_scalar_add(
                        kg[b][:, m2, 392 * n2:392 * n2 + 392],
                        pk[:, :392],
                        shk_sb[:, 2 * g + m2:2 * g + m2 + 1])

            if g == 0 and b == 0:
                # deferred so the first xtb load outruns it on the DMA queue
                nc.sync.dma_start(wq_sb,
                                  wq.rearrange("(c p) n -> p c n", p=128))

            # q for this head group
            for m2 in range(2):
                pq = mmp.tile([128, 512], F32, tag="mm", name="pq")
                for kk in range(3):
                    nc.tensor.matmul(
                        pq[:, :NQ],
                        lhsT=wq_sb[:, kk, 256 * g + 128 * m2:
                                   256 * g + 128 * m2 + 128],
                        rhs=xstb[b][:, kk, :],
                        start=(kk == 0), stop=(kk == 2))
                nc.vector.tensor_scalar_add(
                    qg[b][:, m2, :], pq[:, :NQ],
                    shq_sb[:, 2 * g + m2:2 * g + m2 + 1])

            if g == 0 and b == 0:
                nc.sync.dma_start(wv_sb, wv)
                for h in range(8):
                    nc.sync.dma_start(ebias_sb[:, h, :, :],
                                      ebias[:, h, :, :])
                nc.sync.dma_start(shv_sb, shv)
                nc.sync.dma_start(sel2, sel2d)
                nc.sync.dma_start(idq, idqd)
            if g == 1 and b == 0:
                for h in range(8, 16):
                    nc.sync.dma_start(ebias_sb[:, h, :, :],
                                      ebias[:, h, :, :])

            # v token-major for this head group (512 features), fp8 DR
            for c in range(C):
                pv = mmp.tile([128, 512], F32, tag="mm", name="pv")
                for kp in range(2):
                    nc.tensor.matmul(
                        pv[:MC, :],
                        lhsT=x8tb[b][:, 2 * kp:2 * kp + 2,
                                     MC * c:MC * c + MC],
                        rhs=wv_sb[:, 2 * kp:2 * kp + 2,
                                  512 * g:512 * g + 512],
                        perf_mode=DR,
                        start=(kp == 0), stop=(kp == 1))
                nc.vector.tensor_scalar_mul(vt[b][:, c, :], pv[:MC, :],
                                            1.0 / VSCALE)

        def emit_late_weights():
            nc.sync.dma_start(wp_sb, wp.rearrange("(c p) n -> p c n", p=128))
            nc.sync.dma_start(shp_sb, shp)

        def emit_scores(g, hp, b):
            hhs = (2 * hp, 2 * hp + 1)
            texp2 = [texpp.tile([MC, C, NQ], BF16, tag="texp",
                                name=f"texp{j}") for j in range(2)]
            for cq, banks in ((0, (0, 1)), (1, (2, 3))):
                sc = [None, None]
                for j, hh in enumerate(hhs):
                    h = 8 * g + hh
                    pb = 32 * (hh % 4)
                    m2 = hh // 4
                    sc[j] = scp.tile([MC, 2, 512], F32, tag="sc", name="sc")
                    for bi, bank in enumerate(banks):
                        c0 = 2 * bank
                        if bank < 3:
                            nc.tensor.matmul(
                                sc[j][:, bi, 0:392],
                                lhsT=ident_sb[:, 0, :, :],
                                rhs=ebias_sb[:, h, c0:c0 + 4, :].rearrange(
                                    "p (t c) q -> p t (c q)", t=2),
                                perf_mode=DR, start=True, stop=False,
                                skip_group_check=True)
                        else:
                            nc.tensor.matmul(
                                sc[j][:, bi, 0:196],
                                lhsT=ident_sb[:, 1, :, :],
                                rhs=ebias_sb[:, h, 5:7, :].rearrange(
                                    "p (t c) q -> p t (c q)", t=2),
                                perf_mode=DR, start=True, stop=False,
                                skip_group_check=True)
                        for ci in range(2):
                            c = c0 + ci
                            if c >= C:
                                continue
                            nc.tensor.matmul(
                                sc[j][:, bi, 196 * ci:196 * ci + 196],
                                lhsT=kg[b][pb:pb + 32, m2,
                                           MC * c:MC * c + MC],
                                rhs=qg[b][pb:pb + 32, m2, :],
                                start=False,
                                stop=(ci == 1 or c == C - 1),
                                tile_position=(pb, 0),
                                skip_group_check=True)
                for j in range(2):
                    if cq == 0:
                        nc.scalar.activation(
                            texp2[j][:, 0:4, :].rearrange(
                                "p (a b) q -> p a b q", b=2),
                            sc[j][:, :, 0:392].rearrange(
                                "p a (b q) -> p a b q", q=196),
                            AF.Exp)
                    else:
                        nc.scalar.activation(
                            texp2[j][:, 4:6, :],
                            sc[j][:, 0, 0:392].rearrange(
                                "p (a q) -> p a q", q=196),
                            AF.Exp)
                        nc.scalar.activation(texp2[j][:, 6, :],
                                             sc[j][:, 1, 0:196],
                                             AF.Exp)
            return texp2, hhs

        emitted_A = set()
        state = {"pending": None}

        def flush():
            # attn @ v for the PREVIOUS iteration (one iter of slack), plus
            # the work hooks that interleave with the following scores:
            # after a batch elem's first pair, start the next elem's phase
            # A; after its last pair, start its next-group phase A (g=0) or
            # its output chain (g=1).
            if state["pending"] is None:
                return
            texp2, hhs, pb, pg, php = state["pending"]
            state["pending"] = None
            emit_attn(texp2, hhs, pb, pg)
            if php == 0 and pb + 1 < BC and (pg, pb + 1) not in emitted_A:
                emit_phaseA(pg, pb + 1)
                emitted_A.add((pg, pb + 1))
            if php == HG // 2 - 1:
                if pg == 0:
                    emit_phaseA(1, pb)
                    emitted_A.add((1, pb))
                    if pb == 0:
                        emit_late_weights()
                else:
                    emit_output(pb)

        for g in range(G):
            for b in range(BC):
                if (g, b) not in emitted_A:
                    emit_phaseA(g, b)
                    emitted_A.add((g, b))
                for hp in range(HG // 2):
                    texp2, hhs = emit_scores(g, hp, b)
                    flush()
                    state["pending"] = (texp2, hhs, b, g, hp)
        flush()
    nc.compile()
    return nc


def _prepare_in_maps(inputs):
    inp = {k: np.asarray(v) for k, v in inputs.items()}
    x = inp["x"].astype(np.float32)          # [32, 784, 384]
    Wkv, Wq, Wp = inp["Wkv"], inp["Wq"], inp["Wp"]
    biases, idxs = inp["biases"], inp["idxs"].astype(np.int64)

    s_kv = inp["kv_w"] / np.sqrt(inp["kv_var"] + EPS)
    wkv = (Wkv * s_kv[:, None]).astype(np.float32)
    sh_kv = (inp["kv_b"] - inp["kv_mean"] * s_kv).astype(np.float32)
    wkv3 = wkv.reshape(H, KD + D, IN)
    sh3 = sh_kv.reshape(H, KD + D)
    wkT = np.ascontiguousarray(
        wkv3[:, :KD, :].reshape(H * KD, IN).T.astype(BF16NP))
    sh_k = np.ascontiguousarray(sh3[:, :KD].reshape(H * KD))
    wvT = np.ascontiguousarray(
        wkv3[:, KD:, :].reshape(H * D, IN).T)            # [384, 1024] f32
    sh_v = np.ascontiguousarray(sh3[:, KD:].reshape(H * D))

    s_q = inp["q_w"] / np.sqrt(inp["q_var"] + EPS)
    wqT = np.ascontiguousarray(
        (Wq * (s_q * SCALE)[:, None]).T.astype(BF16NP))
    sh_q = ((inp["q_b"] - inp["q_mean"] * s_q) * SCALE).astype(np.float32)

    s_p = inp["p_w"] / np.sqrt(inp["p_var"] + EPS)
    wpT = np.ascontiguousarray(((Wp * s_p[:, None]) / 6.0).T)  # [1024, 512]
    sh_p = (inp["p_b"] - inp["p_mean"] * s_p).astype(np.float32)

    wv_h = np.zeros((128, 4, DH), np.float32)
    wv_h[:, 0:3, :] = (wvT * VSCALE).reshape(3, 128, DH).transpose(1, 0, 2)
    wv_h = wv_h.astype(FP8NP)
    wp_h = np.ascontiguousarray(wpT.astype(BF16NP))

    shk_h = np.ascontiguousarray(sh_k.reshape(4, 128).T)
    shq_h = np.ascontiguousarray(sh_q.reshape(4, 128).T)
    shv_h = np.ascontiguousarray(sh_v.reshape(8, 128).T)
    shp_h = np.ascontiguousarray(sh_p.reshape(1, OUT))

    # additive bias, gathered, padded to 8 chunks (slot 7 = 0)
    eb = biases.astype(np.float32)[:, idxs]              # [16, 196, 784]
    eb = eb.transpose(0, 2, 1).reshape(H, C, MC, NQ)     # [16, 7, 112, 196]
    eb8 = np.zeros((MC, H, 8, NQ), np.float32)
    eb8[:, :, 0:C, :] = eb.transpose(2, 0, 1, 3)
    eb8 = eb8.astype(FP8NP)

    ident_h = np.zeros((MC, 2, 2, MC), np.float32)
    ident_h[:, 0, 0, :] = np.eye(MC)
    ident_h[:, 1, 1, :] = np.eye(MC)
    ident_h = ident_h.astype(FP8NP)

    sel2_h = np.zeros((16, 8, 128), np.float32)
    for t in range(8):
        sel2_h[2 * t, t, 0:64] = 1.0
        sel2_h[2 * t + 1, t, 64:128] = 1.0
    sel2_h = sel2_h.astype(BF16NP)

    idq_h = np.eye(98, dtype=np.float32)

    xs = x.reshape(B, RES, RES, IN)[:, ::STRIDE, ::STRIDE].reshape(B, NQ, IN)

    shared = {"wk": wkT, "wv": wv_h, "wq": wqT, "wp": wp_h, "shk": shk_h,
              "shq": shq_h, "shv": shv_h, "shp": shp_h, "ebias": eb8,
              "identd": ident_h, "sel2d": sel2_h, "idqd": idq_h}
    in_maps = []
    for i in range(NCORES):
        xb = x[BC * i:BC * i + BC]
        xsb = xs[BC * i:BC * i + BC]
        m = dict(shared)
        xbT = np.ascontiguousarray(xb.transpose(0, 2, 1))   # [BC, 384, 784]
        m["xT"] = xbT.astype(BF16NP)
        x8 = np.zeros((BC, 512, N), np.float32)
        x8[:, 0:IN, :] = xbT
        m["x8T"] = x8.astype(FP8NP)
        m["xsT"] = np.ascontiguousarray(
            xsb.transpose(0, 2, 1)).astype(BF16NP)
        in_maps.append(m)
    return in_maps


def kernel(**inputs):
    global _NC_CACHE, LAST_RESULTS
    in_maps = _prepare_in_maps(inputs)
    if _NC_CACHE is None:
        _NC_CACHE = _build_nc()
    res = run_bass_kernel_spmd(_NC_CACHE, in_maps,
                               core_ids=list(range(NCORES)), trace=TRACE)
    LAST_RESULTS = res
    return np.concatenate([res.results[i]["out"] for i in range(NCORES)],
                          axis=0)


# revision 18
# speedup vs baseline: 1.0097x; 1.0097x over previous
"""AttentionSubsample Trainium2 kernel: 8-core data-parallel over batch.

Layout strategy (per core, 4 batch elements), v3:
  - k/q/scores run in bf16 (fp8 there amplifies through the exp); v and
    the projection run as fp8e4 DoubleRow matmuls (2 K-tiles per
    instruction at 0.5 cycles/row), with x4/x16 pre-scaling so the small
    BN-folded weights stay in fp8's normal range.
  - the relative-position bias is PRELOADED into the score PSUM banks
    with fp8 DoubleRow identity matmuls (I,0)/(0,I), so exp reads (s+b)
    directly and no elementwise bias multiply exists at all.  The whole
    bias table stays resident in SBUF (one DMA), so batch output work
    spreads evenly instead of serializing at the end.
  - attention output per head pair lives in ONE psum bank (even head
    rows 0-63, odd head rows 64-127, each zeroing its rows via its
    chunk-0 start=True); softmax denominators accumulate in columns
    196..199 of the same bank via 1-row ones-matmuls (start=False: the
    attn starts already zeroed the full 2KB bank rows), so the
    denominator needs no 1-partition copy: one [98,4] reciprocal per
    pair reads it q-partitioned.
  - denominator reciprocals are transposed back to head-major [16,196]
    with a PE transpose and broadcast over partitions with a K=16 sel
    matmul; the hardswish chain is split DVE (bias add, clamp) / Pool
    (+3, multiply).
"""

import sys

sys.path.insert(0, "/opt/trn_rl_repo")

from contextlib import ExitStack

import numpy as np
import ml_dtypes

import concourse.bass as bass
import concourse.tile as tile
from concourse import bacc
from concourse import mybir
from concourse.bass_utils import run_bass_kernel_spmd

F32 = mybir.dt.float32
F32R = mybir.dt.float32r
BF16 = mybir.dt.bfloat16
FP8 = mybir.dt.float8e4
ALU = mybir.AluOpType
AF = mybir.ActivationFunctionType
DR = mybir.MatmulPerfMode.DoubleRow

FP8NP = ml_dtypes.float8_e4m3
BF16NP = ml_dtypes.bfloat16

B, N, NQ, IN, H, KD, D, OUT = 32, 784, 196, 384, 16, 32, 64, 512
HID, DH = 1536, 1024
RES, RES_, STRIDE = 28, 14, 2
SCALE = KD ** -0.5
EPS = 1e-5
NCORES = 8
BC = B // NCORES          # 4 batch elems per core
C, MC = 7, 112            # key-token chunks: 7 x 112 = 784
G, HG = 2, 8              # 2 head-groups of 8 heads

VSCALE = 4.0              # fp8 headroom scale on the v weights
PSCALE = 16.0             # fp8 headroom scale on the projection path

TRACE = False
LAST_RESULTS = None

_NC_CACHE = None


def _build_nc():
    nc = bacc.Bacc("TRN2", target_bir_lowering=False, debug=False,
                   num_devices=NCORES)

    xT = nc.dram_tensor("xT", [BC, IN, N], BF16, kind="ExternalInput").ap()
    x8T = nc.dram_tensor("x8T", [BC, 512, N], FP8, kind="ExternalInput").ap()
    xsT = nc.dram_tensor("xsT", [BC, IN, NQ], BF16, kind="ExternalInput").ap()
    wk = nc.dram_tensor("wk", [IN, 512], BF16, kind="ExternalInput").ap()
    wv = nc.dram_tensor("wv", [128, 4, DH], FP8, kind="ExternalInput").ap()
    wq = nc.dram_tensor("wq", [IN, 512], BF16, kind="ExternalInput").ap()
    wp = nc.dram_tensor("wp", [DH, OUT], BF16, kind="ExternalInput").ap()
    shk = nc.dram_tensor("shk", [128, 4], F32, kind="ExternalInput").ap()
    shq = nc.dram_tensor("shq", [128, 4], F32, kind="ExternalInput").ap()
    shv = nc.dram_tensor("shv", [128, 8], F32, kind="ExternalInput").ap()
    shp = nc.dram_tensor("shp", [1, OUT], F32R, kind="ExternalInput").ap()
    ebias = nc.dram_tensor("ebias", [MC, H, 8, NQ], FP8,
                           kind="ExternalInput").ap()
    identd = nc.dram_tensor("identd", [MC, 2, 2, MC], FP8,
                            kind="ExternalInput").ap()
    sel2d = nc.dram_tensor("sel2d", [16, 8, 128], BF16,
                           kind="ExternalInput").ap()
    idqd = nc.dram_tensor("idqd", [98, 98], F32, kind="ExternalInput").ap()
    out = nc.dram_tensor("out", [BC, NQ, OUT], F32, kind="ExternalOutput").ap()

    with tile.TileContext(nc) as tc, ExitStack() as ctx:
        ctx.enter_context(nc.allow_low_precision(
            reason="fp8/bf16 attention path validated against fp32 reference"))
        singles = ctx.enter_context(tc.tile_pool(name="singles", bufs=1))
        texpp = ctx.enter_context(tc.tile_pool(name="texpp", bufs=6))
        tmpp = ctx.enter_context(tc.tile_pool(name="tmpp", bufs=2))
        hswp = ctx.enter_context(tc.tile_pool(name="hswp", bufs=2))
        finp = ctx.enter_context(tc.tile_pool(name="finp", bufs=2))
        mmp = ctx.enter_context(tc.tile_pool(name="mmp", bufs=2, space="PSUM"))
        scp = ctx.enter_context(tc.tile_pool(name="scp", bufs=2, space="PSUM"))
        opp = ctx.enter_context(tc.tile_pool(name="opp", bufs=2, space="PSUM"))

        # --- persistent SBUF ---
        wk_sb = singles.tile([128, 3, 512], BF16)
        nc.sync.dma_start(wk_sb, wk.rearrange("(c p) n -> p c n", p=128))
        wq_sb = singles.tile([128, 3, 512], BF16)
        wv_sb = singles.tile([128, 4, DH], FP8)
        wp_sb = singles.tile([128, 8, OUT], BF16)
        shk_sb = singles.tile([128, 4], F32)
        shq_sb = singles.tile([128, 4], F32)
        shv_sb = singles.tile([128, 8], F32)
        shp_sb = singles.tile([1, OUT], F32R)
        ident_sb = singles.tile([MC, 2, 2, MC], FP8)
        sel2 = singles.tile([16, 8, 128], BF16)
        idq = singles.tile([98, 98], F32)
        ones1 = singles.tile([1, 128], F32)
        nc.gpsimd.memset(ones1, 1.0)
        ones1b = singles.tile([MC, 1], BF16)
        nc.gpsimd.memset(ones1b, 1.0)
        p3t = singles.tile([128, NQ], BF16)
        nc.gpsimd.memset(p3t, 3.0)

        # full bias table resident, loaded in per-head slices so the x
        # loads are not stuck behind one 3.5MB transfer on the DMA queue
        ebias_sb = singles.tile([MC, H, 8, NQ], FP8)

        # per-b persistent tensors
        xtb = [singles.tile([128, 3, N], BF16, name=f"xtb{b}")
               for b in range(BC)]
        x8tb = [singles.tile([128, 4, N], FP8, name=f"x8tb{b}")
                for b in range(BC)]
        xstb = [singles.tile([128, 3, NQ], BF16, name=f"xstb{b}")
                for b in range(BC)]
        kg = [singles.tile([128, 2, N], BF16, name=f"kg{b}")
              for b in range(BC)]
        qg = [singles.tile([128, 2, NQ], BF16, name=f"qg{b}")
              for b in range(BC)]
        vt = [singles.tile([MC, C, 512], BF16, name=f"vt{b}")
              for b in range(BC)]
        acc = [singles.tile([128, 8, NQ], BF16, name=f"acc{b}")
               for b in range(BC)]
        den_q = [singles.tile([98, 2, 16], F32, name=f"denq{b}")
                 for b in range(BC)]
        recsT = [singles.tile([16, NQ], BF16, name=f"recsT{b}")
                 for b in range(BC)]

        pending = None

        def emit_attn(texp2, hhs, b, g):
            t = 4 * g + hhs[0] // 2
            op = opp.tile([128, 512], F32, tag="op", name="op")
            # chunk-0 matmuls with start=True zero each head's full 2KB
            # bank rows (incl. den cols); den matmuls accumulate on that.
            for j in (0, 1):
                hh = hhs[j]
                nc.tensor.matmul(op[64 * j:64 * j + 64, 0:196],
                                 lhsT=vt[b][:, 0, 64 * hh:64 * hh + 64],
                                 rhs=texp2[j][:, 0, :],
                                 start=True, stop=False,
                                 skip_group_check=True)
            for j in (0, 1):
                for qh in (0, 1):
                    cl = 196 + 2 * j + qh
                    for c in range(C):
                        nc.tensor.matmul(
                            op[0:98, cl:cl + 1],
                            lhsT=texp2[j][:, c, 98 * qh:98 * qh + 98],
                            rhs=ones1b,
                            start=False, stop=False, skip_group_check=True)
            for j in (0, 1):
                hh = hhs[j]
                for c in range(1, C):
                    nc.tensor.matmul(op[64 * j:64 * j + 64, 0:196],
                                     lhsT=vt[b][:, c, 64 * hh:64 * hh + 64],
                                     rhs=texp2[j][:, c, :],
                                     start=False,
                                     stop=(j == 1 and c == C - 1),
                                     skip_group_check=True)
            nc.vector.tensor_copy(acc[b][:, t, :], op[:, 0:196])
            nc.vector.reciprocal(
                den_q[b][:, :, 2 * t:2 * t + 2],
                op[0:98, 196:200].rearrange("p (j q) -> p q j", j=2))

        def emit_output(b):
            # transpose the q-partitioned reciprocals to head-major [16,196]
            for qh in (0, 1):
                tr = mmp.tile([128, 512], F32, tag="mm", name="tr")
                nc.tensor.matmul(tr[0:16, 0:98],
                                 lhsT=den_q[b][0:98, qh, :], rhs=idq,
                                 is_transpose=True, start=True, stop=True)
                nc.vector.tensor_copy(recsT[b][:, 98 * qh:98 * qh + 98],
                                      tr[0:16, 0:98])
            hsw = hswp.tile([128, 8, NQ], BF16, tag="hsw", name="hsw")
            for t in range(8):
                rep = mmp.tile([128, 512], F32, tag="mm", name="rep")
                nc.tensor.matmul(rep[:, :NQ], lhsT=sel2[:, t, :],
                                 rhs=recsT[b],
                                 start=True, stop=True)
                t1 = tmpp.tile([128, NQ], BF16, tag="t1", name="t1")
                nc.vector.tensor_tensor(t1, acc[b][:, t, :], rep[:, :NQ],
                                        ALU.mult)
                # vv2 = max(t1 + shv, -3); t3p = min(vv2, 3) + 3
                # hsw = t3p * vv2  (exact: t3p = 0 wherever vv2 != vv)
                vv = tmpp.tile([128, NQ], BF16, tag="vv", name="vv")
                nc.vector.tensor_scalar(vv, t1, shv_sb[:, t:t + 1], -3.0,
                                        ALU.add, ALU.max)
                t3 = tmpp.tile([128, NQ], BF16, tag="t3", name="t3")
                nc.vector.tensor_scalar(t3, vv, 3.0, 3.0, ALU.min, ALU.add)
                nc.gpsimd.tensor_mul(hsw[:, t, :], t3, vv)
            for mt, msz in ((0, 128), (1, 68)):
                po = mmp.tile([128, 512], F32, tag="mm", name="po")
                nc.tensor.matmul(po[:msz, :],
                                 lhsT=ones1.bitcast(F32R)[0:1, 0:msz],
                                 rhs=shp_sb, start=True, stop=False,
                                 skip_group_check=True)
                for kk in range(8):
                    nc.tensor.matmul(
                        po[:msz, :],
                        lhsT=hsw[:, kk, 128 * mt:128 * mt + msz],
                        rhs=wp_sb[:, kk, :], start=False,
                        stop=(kk == 7), skip_group_check=True)
                fin = finp.tile([128, OUT], F32, tag="fin", name="fin")
                nc.vector.tensor_copy(fin[:msz, :], po[:msz, :])
                nc.sync.dma_start(out[b, 128 * mt:128 * mt + msz, :],
                                  fin[:msz, :])

        def emit_phaseA(g, b):
            # ---- phase A: k, q, v for one (group, batch elem) ----
            if g == 0:
                nc.sync.dma_start(xtb[b],
                                  xT[b].rearrange("(c p) n -> p c n", p=128))
                nc.sync.dma_start(x8tb[b],
                                  x8T[b].rearrange("(c p) n -> p c n", p=128))
                nc.sync.dma_start(xstb[b],
                                  xsT[b].rearrange("(c p) n -> p c n", p=128))
            if g == 0 and b == 0:
                nc.sync.dma_start(shk_sb, shk)
                nc.sync.dma_start(ident_sb, identd)
                nc.sync.dma_start(shq_sb, shq)

            # k for this head group: features [256g, 256g+256), feat-major
            for m2 in range(2):
                for n2 in range(2):
                    pk = mmp.tile([128, 512], F32, tag="mm", name="pk")
                    for kk in range(3):
                        nc.tensor.matmul(
                            pk[:, :392],
                            lhsT=wk_sb[:, kk, 256 * g + 128 * m2:
                                       256 * g + 128 * m2 + 128],
                            rhs=xtb[b][:, kk, 392 * n2:392 * n2 + 392],
                            start=(kk == 0), stop=(kk == 2))
                    nc.vector.tensor_scalar_add(
                        kg[b][:, m2, 392 * n2:392 * n2 + 392],
                        pk[:, :392],
                        shk_sb[:, 2 * g + m2:2 * g + m2 + 1])

            if g == 0 and b == 0:
                # deferred so the first xtb load outruns it on the DMA queue
                nc.sync.dma_start(wq_sb,
                                  wq.rearrange("(c p) n -> p c n", p=128))

            # q for this head group
            for m2 in range(2):
                pq = mmp.tile([128, 512], F32, tag="mm", name="pq")
                for kk in range(3):
                    nc.tensor.matmul(
                        pq[:, :NQ],
                        lhsT=wq_sb[:, kk, 256 * g + 128 * m2:
                                   256 * g + 128 * m2 + 128],
                        rhs=xstb[b][:, kk, :],
                        start=(kk == 0), stop=(kk == 2))
                nc.vector.tensor_scalar_add(
                    qg[b][:, m2, :], pq[:, :NQ],
                    shq_sb[:, 2 * g + m2:2 * g + m2 + 1])

            if g == 0 and b == 0:
                nc.sync.dma_start(wv_sb, wv)
                for h in range(8):
                    nc.sync.dma_start(ebias_sb[:, h, :, :],
                                      ebias[:, h, :, :])
                nc.sync.dma_start(shv_sb, shv)
                nc.sync.dma_start(sel2, sel2d)
                nc.sync.dma_start(idq, idqd)
            if g == 1 and b == 0:
                for h in range(8, 16):
                    nc.sync.dma_start(ebias_sb[:, h, :, :],
                                      ebias[:, h, :, :])

            # v token-major for this head group (512 features), fp8 DR
            for c in range(C):
                pv = mmp.tile([128, 512], F32, tag="mm", name="pv")
                for kp in range(2):
                    nc.tensor.matmul(
                        pv[:MC, :],
                        lhsT=x8tb[b][:, 2 * kp:2 * kp + 2,
                                     MC * c:MC * c + MC],
                        rhs=wv_sb[:, 2 * kp:2 * kp + 2,
                                  512 * g:512 * g + 512],
                        perf_mode=DR,
                        start=(kp == 0), stop=(kp == 1))
                nc.vector.tensor_scalar_mul(vt[b][:, c, :], pv[:MC, :],
                                            1.0 / VSCALE)

        def emit_late_weights():
            nc.sync.dma_start(wp_sb, wp.rearrange("(c p) n -> p c n", p=128))
            nc.sync.dma_start(shp_sb, shp)

        def emit_scores(g, hp, b):
            hhs = (2 * hp, 2 * hp + 1)
            texp2 = [texpp.tile([MC, C, NQ], BF16, tag="texp",
                                name=f"texp{j}") for j in range(2)]
            for cq, banks in ((0, (0, 1)), (1, (2, 3))):
                sc = [None, None]
                for j, hh in enumerate(hhs):
                    h = 8 * g + hh
                    pb = 32 * (hh % 4)
                    m2 = hh // 4
                    sc[j] = scp.tile([MC, 2, 512], F32, tag="sc", name="sc")
                    for bi, bank in enumerate(banks):
                        c0 = 2 * bank
                        if bank < 3:
                            nc.tensor.matmul(
                                sc[j][:, bi, 0:392],
                                lhsT=ident_sb[:, 0, :, :],
                                rhs=ebias_sb[:, h, c0:c0 + 4, :].rearrange(
                                    "p (t c) q -> p t (c q)", t=2),
                                perf_mode=DR, start=True, stop=False,
                                skip_group_check=True)
                        else:
                            nc.tensor.matmul(
                                sc[j][:, bi, 0:196],
                                lhsT=ident_sb[:, 1, :, :],
                                rhs=ebias_sb[:, h, 5:7, :].rearrange(
                                    "p (t c) q -> p t (c q)", t=2),
                                perf_mode=DR, start=True, stop=False,
                                skip_group_check=True)
                        for ci in range(2):
                            c = c0 + ci
                            if c >= C:
                                continue
                            nc.tensor.matmul(
                                sc[j][:, bi, 196 * ci:196 * ci + 196],
                                lhsT=kg[b][pb:pb + 32, m2,
                                           MC * c:MC * c + MC],
                                rhs=qg[b][pb:pb + 32, m2, :],
                                start=False,
                                stop=(ci == 1 or c == C - 1),
                                tile_position=(pb, 0),
                                skip_group_check=True)
                for j in range(2):
                    if cq == 0:
                        nc.scalar.activation(
                            texp2[j][:, 0:4, :].rearrange(
                                "p (a b) q -> p a b q", b=2),
                            sc[j][:, :, 0:392].rearrange(
                                "p a (b q) -> p a b q", q=196),
                            AF.Exp)
                    else:
                        nc.scalar.activation(
                            texp2[j][:, 4:6, :],
                            sc[j][:, 0, 0:392].rearrange(
                                "p (a q) -> p a q", q=196),
                            AF.Exp)
                        nc.scalar.activation(texp2[j][:, 6, :],
                                             sc[j][:, 1, 0:196],
                                             AF.Exp)
            return texp2, hhs

        emitted_A = set()
        state = {"pending": None}

        def flush():
            # attn @ v for the PREVIOUS iteration (one iter of slack), plus
            # the work hooks that interleave with the following scores:
            # after a batch elem's first pair, start the next elem's phase
            # A; after its last pair, start its next-group phase A (g=0) or
            # its output chain (g=1).
            if state["pending"] is None:
                return
            texp2, hhs, pb, pg, php = state["pending"]
            state["pending"] = None
            emit_attn(texp2, hhs, pb, pg)
            if php == 0 and pb + 1 < BC and (pg, pb + 1) not in emitted_A:
                emit_phaseA(pg, pb + 1)
                emitted_A.add((pg, pb + 1))
            if php == HG // 2 - 1:
                if pg == 0:
                    emit_phaseA(1, pb)
                    emitted_A.add((1, pb))
                    if pb == 0:
                        emit_late_weights()
                else:
                    emit_output(pb)

        for g in range(G):
            for b in range(BC):
                if (g, b) not in emitted_A:
                    emit_phaseA(g, b)
                    emitted_A.add((g, b))
                for hp in range(HG // 2):
                    texp2, hhs = emit_scores(g, hp, b)
                    flush()
                    state["pending"] = (texp2, hhs, b, g, hp)
        flush()
    nc.compile()
    return nc


def _prepare_in_maps(inputs):
    inp = {k: np.asarray(v) for k, v in inputs.items()}
    x = inp["x"].astype(np.float32)          # [32, 784, 384]
    Wkv, Wq, Wp = inp["Wkv"], inp["Wq"], inp["Wp"]
    biases, idxs = inp["biases"], inp["idxs"].astype(np.int64)

    s_kv = inp["kv_w"] / np.sqrt(inp["kv_var"] + EPS)
    wkv = (Wkv * s_kv[:, None]).astype(np.float32)
    sh_kv = (inp["kv_b"] - inp["kv_mean"] * s_kv).astype(np.float32)
    wkv3 = wkv.reshape(H, KD + D, IN)
    sh3 = sh_kv.reshape(H, KD + D)
    wkT = np.ascontiguousarray(
        wkv3[:, :KD, :].reshape(H * KD, IN).T.astype(BF16NP))
    sh_k = np.ascontiguousarray(sh3[:, :KD].reshape(H * KD))
    wvT = np.ascontiguousarray(
        wkv3[:, KD:, :].reshape(H * D, IN).T)            # [384, 1024] f32
    sh_v = np.ascontiguousarray(sh3[:, KD:].reshape(H * D))

    s_q = inp["q_w"] / np.sqrt(inp["q_var"] + EPS)
    wqT = np.ascontiguousarray(
        (Wq * (s_q * SCALE)[:, None]).T.astype(BF16NP))
    sh_q = ((inp["q_b"] - inp["q_mean"] * s_q) * SCALE).astype(np.float32)

    s_p = inp["p_w"] / np.sqrt(inp["p_var"] + EPS)
    wpT = np.ascontiguousarray(((Wp * s_p[:, None]) / 6.0).T)  # [1024, 512]
    sh_p = (inp["p_b"] - inp["p_mean"] * s_p).astype(np.float32)

    wv_h = np.zeros((128, 4, DH), np.float32)
    wv_h[:, 0:3, :] = (wvT * VSCALE).reshape(3, 128, DH).transpose(1, 0, 2)
    wv_h = wv_h.astype(FP8NP)
    wp_h = np.ascontiguousarray(wpT.astype(BF16NP))

    shk_h = np.ascontiguousarray(sh_k.reshape(4, 128).T)
    shq_h = np.ascontiguousarray(sh_q.reshape(4, 128).T)
    shv_h = np.ascontiguousarray(sh_v.reshape(8, 128).T)
    shp_h = np.ascontiguousarray(sh_p.reshape(1, OUT))

    # additive bias, gathered, padded to 8 chunks (slot 7 = 0)
    eb = biases.astype(np.float32)[:, idxs]              # [16, 196, 784]
    eb = eb.transpose(0, 2, 1).reshape(H, C, MC, NQ)     # [16, 7, 112, 196]
    eb8 = np.zeros((MC, H, 8, NQ), np.float32)
    eb8[:, :, 0:C, :] = eb.transpose(2, 0, 1, 3)
    eb8 = eb8.astype(FP8NP)

    ident_h = np.zeros((MC, 2, 2, MC), np.float32)
    ident_h[:, 0, 0, :] = np.eye(MC)
    ident_h[:, 1, 1, :] = np.eye(MC)
    ident_h = ident_h.astype(FP8NP)

    sel2_h = np.zeros((16, 8, 128), np.float32)
    for t in range(8):
        sel2_h[2 * t, t, 0:64] = 1.0
        sel2_h[2 * t + 1, t, 64:128] = 1.0
    sel2_h = sel2_h.astype(BF16NP)

    idq_h = np.eye(98, dtype=np.float32)

    xs = x.reshape(B, RES, RES, IN)[:, ::STRIDE, ::STRIDE].reshape(B, NQ, IN)

    shared = {"wk": wkT, "wv": wv_h, "wq": wqT, "wp": wp_h, "shk": shk_h,
              "shq": shq_h, "shv": shv_h, "shp": shp_h, "ebias": eb8,
              "identd": ident_h, "sel2d": sel2_h, "idqd": idq_h}
    in_maps = []
    for i in range(NCORES):
        xb = x[BC * i:BC * i + BC]
        xsb = xs[BC * i:BC * i + BC]
        m = dict(shared)
        xbT = np.ascontiguousarray(xb.transpose(0, 2, 1))   # [BC, 384, 784]
        m["xT"] = xbT.astype(BF16NP)
        x8 = np.zeros((BC, 512, N), np.float32)
        x8[:, 0:IN, :] = xbT
        m["x8T"] = x8.astype(FP8NP)
        m["xsT"] = np.ascontiguousarray(
            xsb.transpose(0, 2, 1)).astype(BF16NP)
        in_maps.append(m)
    return in_maps


def kernel(**inputs):
    global _NC_CACHE, LAST_RESULTS
    in_maps = _prepare_in_maps(inputs)
    if _NC_CACHE is None:
        _NC_CACHE = _build_nc()
    res = run_bass_kernel_spmd(_NC_CACHE, in_maps,
                               core_ids=list(range(NCORES)), trace=TRACE)
    LAST_RESULTS = res
    return np.concatenate([res.results[i]["out"] for i in range(NCORES)],
                          axis=0)


# revision 19
# speedup vs baseline: 1.0247x; 1.0149x over previous
"""AttentionSubsample Trainium2 kernel: 8-core data-parallel over batch.

Layout strategy (per core, 4 batch elements), v3:
  - k/q/scores run in bf16 (fp8 there amplifies through the exp); v and
    the projection run as fp8e4 DoubleRow matmuls (2 K-tiles per
    instruction at 0.5 cycles/row), with x4/x16 pre-scaling so the small
    BN-folded weights stay in fp8's normal range.
  - the relative-position bias is PRELOADED into the score PSUM banks
    with fp8 DoubleRow identity matmuls (I,0)/(0,I), so exp reads (s+b)
    directly and no elementwise bias multiply exists at all.  The whole
    bias table stays resident in SBUF (one DMA), so batch output work
    spreads evenly instead of serializing at the end.
  - attention output per head pair lives in ONE psum bank (even head
    rows 0-63, odd head rows 64-127, each zeroing its rows via its
    chunk-0 start=True); softmax denominators accumulate in columns
    196..199 of the same bank via 1-row ones-matmuls (start=False: the
    attn starts already zeroed the full 2KB bank rows), so the
    denominator needs no 1-partition copy: one [98,4] reciprocal per
    pair reads it q-partitioned.
  - denominator reciprocals are transposed back to head-major [16,196]
    with a PE transpose and broadcast over partitions with a K=16 sel
    matmul; the hardswish chain is split DVE (bias add, clamp) / Pool
    (+3, multiply).
"""

import sys

sys.path.insert(0, "/opt/trn_rl_repo")

from contextlib import ExitStack

import numpy as np
import ml_dtypes

import concourse.bass as bass
import concourse.tile as tile
from concourse import bacc
from concourse import mybir
from concourse.bass_utils import run_bass_kernel_spmd

F32 = mybir.dt.float32
F32R = mybir.dt.float32r
BF16 = mybir.dt.bfloat16
FP8 = mybir.dt.float8e4
ALU = mybir.AluOpType
AF = mybir.ActivationFunctionType
DR = mybir.MatmulPerfMode.DoubleRow

FP8NP = ml_dtypes.float8_e4m3
BF16NP = ml_dtypes.bfloat16

B, N, NQ, IN, H, KD, D, OUT = 32, 784, 196, 384, 16, 32, 64, 512
HID, DH = 1536, 1024
RES, RES_, STRIDE = 28, 14, 2
SCALE = KD ** -0.5
EPS = 1e-5
NCORES = 8
BC = B // NCORES          # 4 batch elems per core
C, MC = 7, 112            # key-token chunks: 7 x 112 = 784
G, HG = 2, 8              # 2 head-groups of 8 heads

VSCALE = 4.0              # fp8 headroom scale on the v weights
PSCALE = 16.0             # fp8 headroom scale on the projection path

TRACE = False
LAST_RESULTS = None

_NC_CACHE = None


def _build_nc():
    nc = bacc.Bacc("TRN2", target_bir_lowering=False, debug=False,
                   num_devices=NCORES)

    xT = nc.dram_tensor("xT", [BC, IN, N], BF16, kind="ExternalInput").ap()
    x8T = nc.dram_tensor("x8T", [BC, 512, N], FP8, kind="ExternalInput").ap()
    xsT = nc.dram_tensor("xsT", [BC, IN, NQ], BF16, kind="ExternalInput").ap()
    wk = nc.dram_tensor("wk", [IN, 512], BF16, kind="ExternalInput").ap()
    wv = nc.dram_tensor("wv", [128, 4, DH], FP8, kind="ExternalInput").ap()
    wq = nc.dram_tensor("wq", [IN, 512], BF16, kind="ExternalInput").ap()
    wp = nc.dram_tensor("wp", [DH, OUT], BF16, kind="ExternalInput").ap()
    shk = nc.dram_tensor("shk", [128, 4], F32, kind="ExternalInput").ap()
    shq = nc.dram_tensor("shq", [128, 4], F32, kind="ExternalInput").ap()
    shv = nc.dram_tensor("shv", [128, 8], F32, kind="ExternalInput").ap()
    shp = nc.dram_tensor("shp", [1, OUT], F32R, kind="ExternalInput").ap()
    ebias = nc.dram_tensor("ebias", [MC, H, 8, NQ], FP8,
                           kind="ExternalInput").ap()
    identd = nc.dram_tensor("identd", [MC, 2, 2, MC], FP8,
                            kind="ExternalInput").ap()
    sel2d = nc.dram_tensor("sel2d", [16, 8, 128], BF16,
                           kind="ExternalInput").ap()
    idqd = nc.dram_tensor("idqd", [98, 98], F32, kind="ExternalInput").ap()
    out = nc.dram_tensor("out", [BC, NQ, OUT], F32, kind="ExternalOutput").ap()

    with tile.TileContext(nc) as tc, ExitStack() as ctx:
        ctx.enter_context(nc.allow_low_precision(
            reason="fp8/bf16 attention path validated against fp32 reference"))
        singles = ctx.enter_context(tc.tile_pool(name="singles", bufs=1))
        texpp = ctx.enter_context(tc.tile_pool(name="texpp", bufs=6))
        tmpp = ctx.enter_context(tc.tile_pool(name="tmpp", bufs=2))
        hswp = ctx.enter_context(tc.tile_pool(name="hswp", bufs=2))
        finp = ctx.enter_context(tc.tile_pool(name="finp", bufs=2))
        mmp = ctx.enter_context(tc.tile_pool(name="mmp", bufs=2, space="PSUM"))
        scp = ctx.enter_context(tc.tile_pool(name="scp", bufs=2, space="PSUM"))
        opp = ctx.enter_context(tc.tile_pool(name="opp", bufs=2, space="PSUM"))

        # --- persistent SBUF ---
        wk_sb = singles.tile([128, 3, 512], BF16)
        nc.sync.dma_start(wk_sb, wk.rearrange("(c p) n -> p c n", p=128))
        wq_sb = singles.tile([128, 3, 512], BF16)
        wv_sb = singles.tile([128, 4, DH], FP8)
        wp_sb = singles.tile([128, 8, OUT], BF16)
        shk_sb = singles.tile([128, 4], F32)
        shq_sb = singles.tile([128, 4], F32)
        shv_sb = singles.tile([128, 8], F32)
        shp_sb = singles.tile([1, OUT], F32R)
        ident_sb = singles.tile([MC, 2, 2, MC], FP8)
        sel2 = singles.tile([16, 8, 128], BF16)
        idq = singles.tile([98, 98], F32)
        ones1 = singles.tile([1, 128], F32)
        nc.gpsimd.memset(ones1, 1.0)
        ones1b = singles.tile([MC, 1], BF16)
        nc.gpsimd.memset(ones1b, 1.0)
        p3t = singles.tile([128, NQ], BF16)
        nc.gpsimd.memset(p3t, 3.0)

        # full bias table resident, loaded in per-head slices so the x
        # loads are not stuck behind one 3.5MB transfer on the DMA queue
        ebias_sb = singles.tile([MC, H, 8, NQ], FP8)

        # per-b persistent tensors
        xtb = [singles.tile([128, 3, N], BF16, name=f"xtb{b}")
               for b in range(BC)]
        x8tb = [singles.tile([128, 4, N], FP8, name=f"x8tb{b}")
                for b in range(BC)]
        xstb = [singles.tile([128, 3, NQ], BF16, name=f"xstb{b}")
                for b in range(BC)]
        kg = [singles.tile([128, 2, N], BF16, name=f"kg{b}")
              for b in range(BC)]
        qg = [singles.tile([128, 2, NQ], BF16, name=f"qg{b}")
              for b in range(BC)]
        vt = [singles.tile([MC, C, 512], BF16, name=f"vt{b}")
              for b in range(BC)]
        acc = [singles.tile([128, 8, NQ], BF16, name=f"acc{b}")
               for b in range(BC)]
        den_q = [singles.tile([98, 2, 16], F32, name=f"denq{b}")
                 for b in range(BC)]
        recsT = [singles.tile([16, NQ], BF16, name=f"recsT{b}")
                 for b in range(BC)]

        pending = None

        def emit_attn(texp2, hhs, b, g):
            t = 4 * g + hhs[0] // 2
            op = opp.tile([128, 512], F32, tag="op", name="op")
            # chunk-0 matmuls with start=True zero each head's full 2KB
            # bank rows (incl. den cols); den matmuls accumulate on that.
            for j in (0, 1):
                hh = hhs[j]
                nc.tensor.matmul(op[64 * j:64 * j + 64, 0:196],
                                 lhsT=vt[b][:, 0, 64 * hh:64 * hh + 64],
                                 rhs=texp2[j][:, 0, :],
                                 start=True, stop=False,
                                 skip_group_check=True)
            for j in (0, 1):
                for qh in (0, 1):
                    cl = 196 + 2 * j + qh
                    for c in range(C):
                        nc.tensor.matmul(
                            op[0:98, cl:cl + 1],
                            lhsT=texp2[j][:, c, 98 * qh:98 * qh + 98],
                            rhs=ones1b,
                            start=False, stop=False, skip_group_check=True)
            for j in (0, 1):
                hh = hhs[j]
                for c in range(1, C):
                    nc.tensor.matmul(op[64 * j:64 * j + 64, 0:196],
                                     lhsT=vt[b][:, c, 64 * hh:64 * hh + 64],
                                     rhs=texp2[j][:, c, :],
                                     start=False,
                                     stop=(j == 1 and c == C - 1),
                                     skip_group_check=True)
            nc.vector.tensor_copy(acc[b][:, t, :], op[:, 0:196])
            nc.vector.reciprocal(
                den_q[b][:, :, 2 * t:2 * t + 2],
                op[0:98, 196:200].rearrange("p (j q) -> p q j", j=2))

        def emit_tchain(b, hsw, t):
            rep = mmp.tile([128, 512], F32, tag="mm", name="rep")
            nc.tensor.matmul(rep[:, :NQ], lhsT=sel2[:, t, :],
                             rhs=recsT[b],
                             start=True, stop=True)
            t1 = tmpp.tile([128, NQ], BF16, tag="t1", name="t1")
            nc.vector.tensor_tensor(t1, acc[b][:, t, :], rep[:, :NQ],
                                    ALU.mult)
            # vv2 = max(t1 + shv, -3); t3p = min(vv2, 3) + 3
            # hsw = t3p * vv2  (exact: t3p = 0 wherever vv2 != vv)
            vv = tmpp.tile([128, NQ], BF16, tag="vv", name="vv")
            nc.vector.tensor_scalar(vv, t1, shv_sb[:, t:t + 1], -3.0,
                                    ALU.add, ALU.max)
            t3 = tmpp.tile([128, NQ], BF16, tag="t3", name="t3")
            nc.vector.tensor_scalar(t3, vv, 3.0, 3.0, ALU.min, ALU.add)
            nc.gpsimd.tensor_mul(hsw[:, t, :], t3, vv)

        def emit_output_slices(b):
            # the output chain split into 3 slices so it interleaves with
            # the following iterations' score work instead of lumping
            cell = {}

            def s0():
                for qh in (0, 1):
                    tr = mmp.tile([128, 512], F32, tag="mm", name="tr")
                    nc.tensor.matmul(tr[0:16, 0:98],
                                     lhsT=den_q[b][0:98, qh, :], rhs=idq,
                                     is_transpose=True, start=True, stop=True)
                    nc.vector.tensor_copy(recsT[b][:, 98 * qh:98 * qh + 98],
                                          tr[0:16, 0:98])
                cell["hsw"] = hswp.tile([128, 8, NQ], BF16, tag="hsw",
                                        name="hsw")
                for t in range(4):
                    emit_tchain(b, cell["hsw"], t)

            def s1():
                for t in range(4, 8):
                    emit_tchain(b, cell["hsw"], t)

            def s2():
                hsw = cell["hsw"]
                for mt, msz in ((0, 128), (1, 68)):
                    po = mmp.tile([128, 512], F32, tag="mm", name="po")
                    nc.tensor.matmul(po[:msz, :],
                                     lhsT=ones1.bitcast(F32R)[0:1, 0:msz],
                                     rhs=shp_sb, start=True, stop=False,
                                     skip_group_check=True)
                    for kk in range(8):
                        nc.tensor.matmul(
                            po[:msz, :],
                            lhsT=hsw[:, kk, 128 * mt:128 * mt + msz],
                            rhs=wp_sb[:, kk, :], start=False,
                            stop=(kk == 7), skip_group_check=True)
                    fin = finp.tile([128, OUT], F32, tag="fin", name="fin")
                    nc.vector.tensor_copy(fin[:msz, :], po[:msz, :])
                    nc.sync.dma_start(out[b, 128 * mt:128 * mt + msz, :],
                                      fin[:msz, :])

            return [s0, s1, s2]

        def emit_phaseA(g, b):
            # ---- phase A: k, q, v for one (group, batch elem) ----
            if g == 0:
                nc.sync.dma_start(xtb[b],
                                  xT[b].rearrange("(c p) n -> p c n", p=128))
                nc.sync.dma_start(x8tb[b],
                                  x8T[b].rearrange("(c p) n -> p c n", p=128))
                nc.sync.dma_start(xstb[b],
                                  xsT[b].rearrange("(c p) n -> p c n", p=128))
            if g == 0 and b == 0:
                nc.sync.dma_start(shk_sb, shk)
                nc.sync.dma_start(ident_sb, identd)
                nc.sync.dma_start(shq_sb, shq)

            # k for this head group: features [256g, 256g+256), feat-major
            for m2 in range(2):
                for n2 in range(2):
                    pk = mmp.tile([128, 512], F32, tag="mm", name="pk")
                    for kk in range(3):
                        nc.tensor.matmul(
                            pk[:, :392],
                            lhsT=wk_sb[:, kk, 256 * g + 128 * m2:
                                       256 * g + 128 * m2 + 128],
                            rhs=xtb[b][:, kk, 392 * n2:392 * n2 + 392],
                            start=(kk == 0), stop=(kk == 2))
                    nc.vector.tensor_scalar_add(
                        kg[b][:, m2, 392 * n2:392 * n2 + 392],
                        pk[:, :392],
                        shk_sb[:, 2 * g + m2:2 * g + m2 + 1])

            if g == 0 and b == 0:
                # deferred so the first xtb load outruns it on the DMA queue
                nc.sync.dma_start(wq_sb,
                                  wq.rearrange("(c p) n -> p c n", p=128))

            # q for this head group
            for m2 in range(2):
                pq = mmp.tile([128, 512], F32, tag="mm", name="pq")
                for kk in range(3):
                    nc.tensor.matmul(
                        pq[:, :NQ],
                        lhsT=wq_sb[:, kk, 256 * g + 128 * m2:
                                   256 * g + 128 * m2 + 128],
                        rhs=xstb[b][:, kk, :],
                        start=(kk == 0), stop=(kk == 2))
                nc.vector.tensor_scalar_add(
                    qg[b][:, m2, :], pq[:, :NQ],
                    shq_sb[:, 2 * g + m2:2 * g + m2 + 1])

            if g == 0 and b == 0:
                nc.sync.dma_start(wv_sb, wv)
                for h in range(8):
                    nc.sync.dma_start(ebias_sb[:, h, :, :],
                                      ebias[:, h, :, :])
                nc.sync.dma_start(shv_sb, shv)
                nc.sync.dma_start(sel2, sel2d)
                nc.sync.dma_start(idq, idqd)
            if g == 1 and b == 0:
                for h in range(8, 16):
                    nc.sync.dma_start(ebias_sb[:, h, :, :],
                                      ebias[:, h, :, :])

            # v token-major for this head group (512 features), fp8 DR
            for c in range(C):
                pv = mmp.tile([128, 512], F32, tag="mm", name="pv")
                for kp in range(2):
                    nc.tensor.matmul(
                        pv[:MC, :],
                        lhsT=x8tb[b][:, 2 * kp:2 * kp + 2,
                                     MC * c:MC * c + MC],
                        rhs=wv_sb[:, 2 * kp:2 * kp + 2,
                                  512 * g:512 * g + 512],
                        perf_mode=DR,
                        start=(kp == 0), stop=(kp == 1))
                nc.vector.tensor_scalar_mul(vt[b][:, c, :], pv[:MC, :],
                                            1.0 / VSCALE)

        def emit_late_weights():
            nc.sync.dma_start(wp_sb, wp.rearrange("(c p) n -> p c n", p=128))
            nc.sync.dma_start(shp_sb, shp)

        def emit_scores(g, hp, b):
            hhs = (2 * hp, 2 * hp + 1)
            texp2 = [texpp.tile([MC, C, NQ], BF16, tag="texp",
                                name=f"texp{j}") for j in range(2)]
            for cq, banks in ((0, (0, 1)), (1, (2, 3))):
                sc = [None, None]
                for j, hh in enumerate(hhs):
                    h = 8 * g + hh
                    pb = 32 * (hh % 4)
                    m2 = hh // 4
                    sc[j] = scp.tile([MC, 2, 512], F32, tag="sc", name="sc")
                    for bi, bank in enumerate(banks):
                        c0 = 2 * bank
                        if bank < 3:
                            nc.tensor.matmul(
                                sc[j][:, bi, 0:392],
                                lhsT=ident_sb[:, 0, :, :],
                                rhs=ebias_sb[:, h, c0:c0 + 4, :].rearrange(
                                    "p (t c) q -> p t (c q)", t=2),
                                perf_mode=DR, start=True, stop=False,
                                skip_group_check=True)
                        else:
                            nc.tensor.matmul(
                                sc[j][:, bi, 0:196],
                                lhsT=ident_sb[:, 1, :, :],
                                rhs=ebias_sb[:, h, 5:7, :].rearrange(
                                    "p (t c) q -> p t (c q)", t=2),
                                perf_mode=DR, start=True, stop=False,
                                skip_group_check=True)
                        for ci in range(2):
                            c = c0 + ci
                            if c >= C:
                                continue
                            nc.tensor.matmul(
                                sc[j][:, bi, 196 * ci:196 * ci + 196],
                                lhsT=kg[b][pb:pb + 32, m2,
                                           MC * c:MC * c + MC],
                                rhs=qg[b][pb:pb + 32, m2, :],
                                start=False,
                                stop=(ci == 1 or c == C - 1),
                                tile_position=(pb, 0),
                                skip_group_check=True)
                for j in range(2):
                    if cq == 0:
                        nc.scalar.activation(
                            texp2[j][:, 0:4, :].rearrange(
                                "p (a b) q -> p a b q", b=2),
                            sc[j][:, :, 0:392].rearrange(
                                "p a (b q) -> p a b q", q=196),
                            AF.Exp)
                    else:
                        nc.scalar.activation(
                            texp2[j][:, 4:6, :],
                            sc[j][:, 0, 0:392].rearrange(
                                "p (a q) -> p a q", q=196),
                            AF.Exp)
                        nc.scalar.activation(texp2[j][:, 6, :],
                                             sc[j][:, 1, 0:196],
                                             AF.Exp)
            return texp2, hhs

        emitted_A = set()
        state = {"pending": None, "outq": []}

        def flush():
            # attn @ v for the PREVIOUS iteration (one iter of slack), plus
            # the work hooks that interleave with the following scores:
            # after a batch elem's first pair, start the next elem's phase
            # A; after its last pair, start its next-group phase A (g=0) or
            # its output chain (g=1).
            if state["pending"] is None:
                return
            texp2, hhs, pb, pg, php = state["pending"]
            state["pending"] = None
            emit_attn(texp2, hhs, pb, pg)
            if php == 0 and pb + 1 < BC and (pg, pb + 1) not in emitted_A:
                emit_phaseA(pg, pb + 1)
                emitted_A.add((pg, pb + 1))
            if php == HG // 2 - 1:
                if pg == 0:
                    emit_phaseA(1, pb)
                    emitted_A.add((1, pb))
                    if pb == 0:
                        emit_late_weights()
                else:
                    state["outq"].extend(emit_output_slices(pb))
            if state["outq"]:
                state["outq"].pop(0)()

        for g in range(G):
            for b in range(BC):
                if (g, b) not in emitted_A:
                    emit_phaseA(g, b)
                    emitted_A.add((g, b))
                for hp in range(HG // 2):
                    texp2, hhs = emit_scores(g, hp, b)
                    flush()
                    state["pending"] = (texp2, hhs, b, g, hp)
        flush()
        while state["outq"]:
            state["outq"].pop(0)()
    nc.compile()
    return nc


def _prepare_in_maps(inputs):
    inp = {k: np.asarray(v) for k, v in inputs.items()}
    x = inp["x"].astype(np.float32)          # [32, 784, 384]
    Wkv, Wq, Wp = inp["Wkv"], inp["Wq"], inp["Wp"]
    biases, idxs = inp["biases"], inp["idxs"].astype(np.int64)

    s_kv = inp["kv_w"] / np.sqrt(inp["kv_var"] + EPS)
    wkv = (Wkv * s_kv[:, None]).astype(np.float32)
    sh_kv = (inp["kv_b"] - inp["kv_mean"] * s_kv).astype(np.float32)
    wkv3 = wkv.reshape(H, KD + D, IN)
    sh3 = sh_kv.reshape(H, KD + D)
    wkT = np.ascontiguousarray(
        wkv3[:, :KD, :].reshape(H * KD, IN).T.astype(BF16NP))
    sh_k = np.ascontiguousarray(sh3[:, :KD].reshape(H * KD))
    wvT = np.ascontiguousarray(
        wkv3[:, KD:, :].reshape(H * D, IN).T)            # [384, 1024] f32
    sh_v = np.ascontiguousarray(sh3[:, KD:].reshape(H * D))

    s_q = inp["q_w"] / np.sqrt(inp["q_var"] + EPS)
    wqT = np.ascontiguousarray(
        (Wq * (s_q * SCALE)[:, None]).T.astype(BF16NP))
    sh_q = ((inp["q_b"] - inp["q_mean"] * s_q) * SCALE).astype(np.float32)

    s_p = inp["p_w"] / np.sqrt(inp["p_var"] + EPS)
    wpT = np.ascontiguousarray(((Wp * s_p[:, None]) / 6.0).T)  # [1024, 512]
    sh_p = (inp["p_b"] - inp["p_mean"] * s_p).astype(np.float32)

    wv_h = np.zeros((128, 4, DH), np.float32)
    wv_h[:, 0:3, :] = (wvT * VSCALE).reshape(3, 128, DH).transpose(1, 0, 2)
    wv_h = wv_h.astype(FP8NP)
    wp_h = np.ascontiguousarray(wpT.astype(BF16NP))

    shk_h = np.ascontiguousarray(sh_k.reshape(4, 128).T)
    shq_h = np.ascontiguousarray(sh_q.reshape(4, 128).T)
    shv_h = np.ascontiguousarray(sh_v.reshape(8, 128).T)
    shp_h = np.ascontiguousarray(sh_p.reshape(1, OUT))

    # additive bias, gathered, padded to 8 chunks (slot 7 = 0)
    eb = biases.astype(np.float32)[:, idxs]              # [16, 196, 784]
    eb = eb.transpose(0, 2, 1).reshape(H, C, MC, NQ)     # [16, 7, 112, 196]
    eb8 = np.zeros((MC, H, 8, NQ), np.float32)
    eb8[:, :, 0:C, :] = eb.transpose(2, 0, 1, 3)
    eb8 = eb8.astype(FP8NP)

    ident_h = np.zeros((MC, 2, 2, MC), np.float32)
    ident_h[:, 0, 0, :] = np.eye(MC)
    ident_h[:, 1, 1, :] = np.eye(MC)
    ident_h = ident_h.astype(FP8NP)

    sel2_h = np.zeros((16, 8, 128), np.float32)
    for t in range(8):
        sel2_h[2 * t, t, 0:64] = 1.0
        sel2_h[2 * t + 1, t, 64:128] = 1.0
    sel2_h = sel2_h.astype(BF16NP)

    idq_h = np.eye(98, dtype=np.float32)

    xs = x.reshape(B, RES, RES, IN)[:, ::STRIDE, ::STRIDE].reshape(B, NQ, IN)

    shared = {"wk": wkT, "wv": wv_h, "wq": wqT, "wp": wp_h, "shk": shk_h,
              "shq": shq_h, "shv": shv_h, "shp": shp_h, "ebias": eb8,
              "identd": ident_h, "sel2d": sel2_h, "idqd": idq_h}
    in_maps = []
    for i in range(NCORES):
        xb = x[BC * i:BC * i + BC]
        xsb = xs[BC * i:BC * i + BC]
        m = dict(shared)
        xbT = np.ascontiguousarray(xb.transpose(0, 2, 1))   # [BC, 384, 784]
        m["xT"] = xbT.astype(BF16NP)
        x8 = np.zeros((BC, 512, N), np.float32)
        x8[:, 0:IN, :] = xbT
        m["x8T"] = x8.astype(FP8NP)
        m["xsT"] = np.ascontiguousarray(
            xsb.transpose(0, 2, 1)).astype(BF16NP)
        in_maps.append(m)
    return in_maps


def kernel(**inputs):
    global _NC_CACHE, LAST_RESULTS
    in_maps = _prepare_in_maps(inputs)
    if _NC_CACHE is None:
        _NC_CACHE = _build_nc()
    res = run_bass_kernel_spmd(_NC_CACHE, in_maps,
                               core_ids=list(range(NCORES)), trace=TRACE)
    LAST_RESULTS = res
    return np.concatenate([res.results[i]["out"] for i in range(NCORES)],
                          axis=0)
